# revision 1
# baseline (speedup 1.0000x reference)
"""Trainium2 Bass kernel for a 2-layer GRU (B=4096, T=128, D=32, H=64) + linear head.

Strategy
--------
Data-parallel over batch: B=4096 -> 8 NeuronCores x 512. Each core runs the
full T=128 recurrence for its batch shard. Layout on chip is "gate-major":
activations live as [gates/hidden on partitions, batch on the free dim], so
the recurrent matmuls are `W^T (stationary) x state (moving)` with N=512
streamed columns and all elementwise work has free-dim 512.

Per GRU step (layer l, input src [K,512], state [64,512]):
  psum_rz[128,512]  = Wx_rz^T src + Wh_rz^T state          (PE, accumulated)
  rz                = sigmoid(psum_rz + b_rz)              (ACT, bias folded)
  t                 = (psum_hn + b_hh_n) * r               (DVE scalar_tensor_tensor)
  psum_xn          += I64 @ t                              (PE identity-accumulate)
  n                 = tanh(psum_xn + b_ih_n)               (ACT, bias folded)
  d = state - n     (GPSIMD)   e = z*d  (DVE)   state' = n + e  (DVE)

The two GRU layers are pipelined one step apart (wavefront), so the
sequential per-step dependency chain of one layer overlaps with the other
layer's work on every engine.
"""

import sys

if "/opt/trn_rl_repo" not in sys.path:
    sys.path.insert(0, "/opt/trn_rl_repo")

import numpy as np
import ml_dtypes

B, T, D, H = 4096, 128, 32, 64
NCORES = 8
BL = B // NCORES  # per-core batch = 512
G3 = 3 * H        # 192 gates, order [r | z | n]

_CACHE = {}


def _legalize_sync(nc, mybir):
    """Split per-instruction semaphore waits that exceed the ISA wait-slot
    budget into EventSemaphore instructions on the same engine queue.

    This walrus build enforces (empirically): compute engines (ACT/DVE/Pool)
    1 wait, DMA 2, PE matmul 3, Drain/CTRL 2. Tile's scheduler freely attaches
    more; excess waits are moved to wait-only EVSEMs issued immediately
    before, which the engine sequencer executes in order — identical
    semantics, legal encoding.
    """
    budget = {
    }  # every instruction type: 1 wait max (walrus adds internal waits)
    ctr = 0
    for f in nc.m.functions:
        for blk in f.blocks:
            out = []
            changed = False
            for inst in blk.instructions:
                si = inst.sync_info
                waits = list(si.on_wait) if (si is not None and si.on_wait) else []
                b = budget.get(type(inst).__name__, 1)
                if len(waits) > b:
                    excess, keep = waits[:-b], waits[-b:]
                    for w in excess:
                        ctr += 1
                        out.append(
                            mybir.InstEventSemaphore(
                                name=f"evw{ctr}_{inst.name}",
                                engine=inst.engine,
                                ins=[],
                                outs=[],
                                sync_info=mybir.SyncInfo(on_wait=[w], on_update=[]),
                            )
                        )
                    si.on_wait = keep
                    changed = True
                out.append(inst)
            if changed:
                try:
                    blk.instructions = out
                except Exception:
                    blk.instructions.clear()
                    blk.instructions.extend(out)
    return ctr


def build_module(t_steps=T, bl=BL, reps=1):
    """Build the Bass module (single program, run SPMD on 8 cores).

    reps>1 repeats the whole wavefront (same x) for slope-timing the real
    device execution under the ~80ms axon dispatch overhead.
    """
    from contextlib import ExitStack

    import concourse.bass as bass
    import concourse.tile as tile
    from concourse import mybir

    f32 = mybir.dt.float32
    bf16 = mybir.dt.bfloat16
    AF = mybir.ActivationFunctionType
    OP = mybir.AluOpType

    nc = bass.Bass()

    # ---- DRAM I/O (per-core shapes) ----
    # All small constants are host-packed into two tensors so they arrive in
    # two DMAs (one semaphore source each) — per-instruction wait slots are a
    # scarce HW resource (setupSyncWait limit).
    CW = 840  # bf16 const pack width
    x_d = nc.dram_tensor("x", [t_steps, D, bl], bf16, kind="ExternalInput")
    cb_d = nc.dram_tensor("cb", [128, CW], bf16, kind="ExternalInput")
    cf_d = nc.dram_tensor("cf", [128, 8], f32, kind="ExternalInput")
    out_d = nc.dram_tensor("out", [1, bl], f32, kind="ExternalOutput")

    with ExitStack() as ctx:
        tc = ctx.enter_context(tile.TileContext(nc))
        const = ctx.enter_context(tc.tile_pool(name="const", bufs=1))
        xpool = ctx.enter_context(tc.tile_pool(name="xp", bufs=6))
        spool = ctx.enter_context(tc.tile_pool(name="state", bufs=8))
        work = ctx.enter_context(tc.tile_pool(name="work", bufs=8))
        ps_rz = ctx.enter_context(tc.tile_pool(name="ps_rz", bufs=2, space="PSUM"))
        ps_xn = ctx.enter_context(tc.tile_pool(name="ps_xn", bufs=3, space="PSUM"))
        ps_hn = ctx.enter_context(tc.tile_pool(name="ps_hn", bufs=3, space="PSUM"))

        # ---- constants in SBUF (two packed tiles, two DMAs) ----
        cb = const.tile([128, CW], bf16, tag="cb")
        nc.sync.dma_start(out=cb, in_=cb_d[:])
        cf = const.tile([128, 8], f32, tag="cf")
        nc.sync.dma_start(out=cf, in_=cf_d[:])
        wx_sb = [cb[0:D, 0:G3], cb[0:H, G3 : 2 * G3]]
        wh_sb = [cb[0:H, 2 * G3 : 3 * G3], cb[0:H, 3 * G3 : 4 * G3]]
        ident = cb[:, 4 * G3 : 4 * G3 + H]  # identity at partitions 64:128
        fcw_sb = cb[0:H, 4 * G3 + H : 4 * G3 + H + 1]
        brz_sb = [cf[:, 0:1], cf[:, 1:2]]
        bni_sb = [cf[0:H, 2:3], cf[0:H, 3:4]]
        bnh_sb = [cf[:, 4:5], cf[:, 5:6]]
        fcb_sb = cf[0:1, 6:7]

        # ACT warm-up: absorbs the sigmoid/tanh table-load and the cf DMA
        # wait into an instruction with spare wait slots (ACT wait-slot limit).
        warm = work.tile([128, 8], f32, tag="warm")
        nc.scalar.activation(warm, cf, AF.Sigmoid)
        warm_v = work.tile([128, 8], f32, tag="warm_v")
        nc.vector.tensor_copy(warm_v, cf)

        # Preload all of x: 8 chunk tiles written once each (no WAR/WAW waits
        # on the hot path; consumers wait on one DMA sem per 16 steps).
        CH = max(1, t_steps // 8)
        x_chunks = []
        for c in range(0, t_steps, CH):
            n_t = min(CH, t_steps - c)
            xc = const.tile([D, n_t, bl], bf16, tag=f"xc{c}")
            nc.sync.dma_start(
                out=xc, in_=x_d[c : c + n_t].rearrange("t d b -> d t b")
            )
            x_chunks.append(xc)

        def x_slice(s):
            return x_chunks[s // CH][:, s % CH, :]

        def gru_step(l, src, state_prev):
            """Emit one GRU step; returns the new state tile [H, bl] bf16.

            Gate order is [z | r | n] (host pre-permuted): z at partitions
            0:64 aligns with the h-space tensors (state/n/d/e, base 0);
            r at partitions 64:128 aligns with hn/t (base 64), so every
            SBUF-SBUF tensor_tensor has equal start partitions.
            """
            prz = ps_rz.tile([2 * H, bl], f32, tag="rz")
            nc.tensor.matmul(prz, lhsT=wx_sb[l][:, 0 : 2 * H], rhs=src,
                             start=True, stop=False)
            nc.tensor.matmul(prz, lhsT=wh_sb[l][:, 0 : 2 * H], rhs=state_prev,
                             start=False, stop=True)
            # xn -> partitions 0:64 of its bank; hn -> partitions 64:128
            pxn = ps_xn.tile([2 * H, bl], f32, tag="xn")
            nc.tensor.matmul(pxn[0:H, :], lhsT=wx_sb[l][:, 2 * H : G3], rhs=src,
                             start=True, stop=False, skip_group_check=True)
            phn = ps_hn.tile([2 * H, bl], f32, tag="hn")
            nc.tensor.matmul(phn[H : 2 * H, :], lhsT=wh_sb[l][:, 2 * H : G3],
                             rhs=state_prev, start=True, stop=True)

            rz = work.tile([2 * H, bl], bf16, tag="rz_s")
            nc.scalar.activation(rz, prz, AF.Sigmoid, bias=brz_sb[l])

            # t = (hn + b_hh_n) * r   on lanes 64:128
            t = work.tile([2 * H, bl], bf16, tag="t")
            nc.vector.scalar_tensor_tensor(
                out=t[H : 2 * H, :], in0=phn[H : 2 * H, :],
                scalar=bnh_sb[l][H : 2 * H, :], in1=rz[H : 2 * H, :],
                op0=OP.add, op1=OP.mult)

            # psum_xn[0:64] += t  (identity stationary at rows 64:128)
            nc.tensor.matmul(pxn[0:H, :], lhsT=ident[H : 2 * H, :],
                             rhs=t[H : 2 * H, :], start=False, stop=True,
                             skip_group_check=True)

            n = work.tile([H, bl], bf16, tag="n")
            nc.scalar.activation(n, pxn[0:H, :], AF.Tanh, bias=bni_sb[l])

            d = work.tile([H, bl], bf16, tag="d")
            nc.gpsimd.tensor_sub(d, state_prev, n)
            e = work.tile([H, bl], bf16, tag="e")
            nc.vector.tensor_mul(e, rz[0:H, :], d)
            ns = spool.tile([H, bl], bf16, tag=("g" if l == 0 else "h"))
            nc.vector.tensor_add(ns, n, e)
            return ns

        g_prev = spool.tile([H, bl], bf16, tag="g")
        h_prev = spool.tile([H, bl], bf16, tag="h")
        nc.vector.memset(g_prev, 0.0)
        nc.vector.memset(h_prev, 0.0)

        n_steps = t_steps * reps
        g_list = [None] * (n_steps + 1)
        g_list[0] = g_prev  # g_list[s+1] = layer-0 output at step s

        for s in range(n_steps + 1):
            if s < n_steps:
                g_list[s + 1] = gru_step(0, x_slice(s % t_steps), g_list[s])
            if s >= 1:
                # layer 1, step s-1 consumes layer-0 output of step s-1
                h_prev = gru_step(1, g_list[s], h_prev)

        # final projection: out = fc_w @ h_T + fc_b   -> [1, bl]
        pfc = ps_rz.tile([1, bl], f32, tag="rz")
        nc.tensor.matmul(pfc, lhsT=fcw_sb, rhs=h_prev, start=True, stop=True)
        out_sb = work.tile([1, bl], f32, tag="out")
        nc.scalar.activation(out_sb, pfc, AF.Identity, bias=fcb_sb)
        nc.sync.dma_start(out=out_d[:], in_=out_sb)

    _legalize_sync(nc, mybir)
    return nc


def shard_inputs(inputs, bl=BL, ncores=NCORES, t_steps=T):
    """Host-side prep: transpose/cast/shard full inputs into per-core maps."""
    bf = ml_dtypes.bfloat16
    x = np.asarray(inputs["x"], dtype=np.float32)
    xT = np.ascontiguousarray(x[: bl * ncores, :t_steps, :].transpose(1, 2, 0)).astype(bf)

    def wT(w):
        return np.ascontiguousarray(np.asarray(w, dtype=np.float32).T).astype(bf)

    def gates_zrn(w):
        """Permute gate rows [r|z|n] -> [z|r|n], then transpose to [in, 3H]."""
        w = np.asarray(w, dtype=np.float32)
        w = np.concatenate([w[H : 2 * H], w[0:H], w[2 * H :]], axis=0)
        return np.ascontiguousarray(w.T).astype(bf)

    CW = 840
    cb = np.zeros((128, CW), dtype=bf)
    cb[0:D, 0:G3] = gates_zrn(inputs["W_ih0"])
    cb[0:H, G3 : 2 * G3] = gates_zrn(inputs["W_ih1"])
    cb[0:H, 2 * G3 : 3 * G3] = gates_zrn(inputs["W_hh0"])
    cb[0:H, 3 * G3 : 4 * G3] = gates_zrn(inputs["W_hh1"])
    cb[H:128, 4 * G3 : 4 * G3 + H] = np.eye(H, dtype=np.float32).astype(bf)
    cb[0:H, 4 * G3 + H] = wT(inputs["fc_w"]).reshape(H)

    cf = np.zeros((128, 8), dtype=np.float32)
    for l in range(2):
        bi = np.asarray(inputs[f"b_ih{l}"], dtype=np.float32)
        bh = np.asarray(inputs[f"b_hh{l}"], dtype=np.float32)
        bzr = bi[: 2 * H] + bh[: 2 * H]
        cf[:, l] = np.concatenate([bzr[H:], bzr[:H]])  # [z | r] order
        cf[0:H, 2 + l] = bi[2 * H :]
        cf[H:128, 4 + l] = bh[2 * H :]
    cf[0, 6] = np.asarray(inputs["fc_b"], dtype=np.float32).reshape(())

    shared = {"cb": cb, "cf": cf}

    in_maps = []
    for c in range(ncores):
        m = dict(shared)
        m["x"] = np.ascontiguousarray(xT[:, :, c * bl : (c + 1) * bl])
        in_maps.append(m)
    return in_maps


def kernel(**inputs):
    from concourse import bass_utils

    if "nc" not in _CACHE:
        _CACHE["nc"] = build_module()
    nc = _CACHE["nc"]
    in_maps = shard_inputs(inputs)
    res = bass_utils.run_bass_kernel_spmd(nc, in_maps, core_ids=list(range(NCORES)))
    out = np.concatenate([r["out"].reshape(BL) for r in res.results])
    return out.astype(np.float32)

